# revision 14
# baseline (speedup 1.0000x reference)
"""GraphSAGE (3-layer, mean-agg) on 8 Trainium2 NeuronCores.

Strategy (matches the sharding hint):
  - Nodes sharded by id range across 8 cores; 256x256 weights replicated;
    edges partitioned by destination-node owner.
  - Node space is PERMUTED into (core, group, slot) blocks: each core's
    6250 destinations are greedily packed into G groups of <=128 distinct
    dsts whose edges fit 1024 slots per src-half. Every group then owns a
    static, disjoint 128-row block of the layout — no scatter needed.
  - Per layer, per core, per group: dma_gather pulls h[src] rows (f32,
    1KB descriptors, 4 SWDGE queues) from a full replica of h in local
    HBM; host-built one-hot segment matrices S (inv_deg folded in) turn
    the segment-sum into TensorE matmuls accumulated in PSUM; the dense
    layer (y^T = Wl^T @ agg^T + Wr^T @ h^T + b, relu) is fused right
    after, with stationary weight tiles; y rows land at the group's
    static block in the shard buffer.
  - After layers 0 and 1 an AllGather rebuilds the full (permuted)
    replica. Output is un-permuted on the host.
"""

import sys

sys.path.insert(0, "/opt/trn_rl_repo")

import numpy as np
import ml_dtypes

import concourse.bass as bass
import concourse.bacc as bacc
import concourse.tile as tile
import concourse.mybir as mybir
from concourse.bass_utils import run_bass_kernel_spmd

BF16 = ml_dtypes.bfloat16

N = 50000
E = 800000
D = 256
L = 3
P = 8
NSH = N // P            # 6250 nodes per core
CAPB = 8                # gather blocks (of 128 edges) per src-half per group
CAP = CAPB * 128        # 1024 edge slots per src-half per group
NBLK = 2 * CAPB         # 16 segment matmul blocks per group


def _pack_idx16(idx):
    """Pack idx list (len multiple of 16) into [128, len/16] int16 layout:
    slot j -> [j % 16, j // 16], replicated to all 8 Q7-core stripes."""
    n = idx.shape[0]
    return np.tile(idx.reshape(n // 16, 16).T, (8, 1)).astype(np.int16)


def _preprocess(x, edge_index):
    """Group edges by dst windows; build permuted layout + gather/segment
    tables. Returns dict of host arrays + layout info."""
    src = edge_index[0].astype(np.int64)
    dst = edge_index[1].astype(np.int64)
    deg = np.bincount(dst, minlength=N).astype(np.float64)
    inv_deg = (1.0 / np.maximum(deg, 1.0)).astype(np.float32)

    # pass 1: pack groups using an approximate chunk-half split of srcs
    # (true split is by local row < GPH, only known after grouping) with a
    # capacity margin to absorb reclassification drift.
    CAP_EFF = CAP - 48
    approx_isB = (src % NSH) >= (NSH // 2)
    cores = []
    for c in range(P):
        lo, hi = c * NSH, (c + 1) * NSH
        m = (dst >= lo) & (dst < hi)
        s_c = src[m]
        d_c = dst[m] - lo
        order = np.argsort(d_c, kind="stable")
        s_c, d_c = s_c[order], d_c[order]
        isB = approx_isB[m][order]
        degA = np.bincount(d_c[~isB], minlength=NSH)
        degB = np.bincount(d_c[isB], minlength=NSH)
        assert degA.max() <= CAP_EFF and degB.max() <= CAP_EFF

        groups = []  # (base, end)
        base, ca, cb = 0, 0, 0
        for dd in range(NSH):
            da, db = degA[dd], degB[dd]
            if (ca + da > CAP_EFF) or (cb + db > CAP_EFF) or (dd - base >= 128):
                groups.append((base, dd))
                base, ca, cb = dd, 0, 0
            ca += da
            cb += db
        groups.append((base, NSH))
        cores.append((groups, s_c, d_c))

    G = max(len(g[0]) for g in cores)
    if G % 2:
        G += 1              # even so the chunk boundary is a group boundary
    GP = G * 128            # local rows per core
    GPH = GP // 2           # chunk-half rows per core
    NP = P * GP
    PHALF = P * GPH         # global rows in chunk 1
    assert PHALF < 32768 and NP - PHALF < 32768

    # node id -> local row; node id -> global (chunked) row
    lperm = np.full(N, -1, dtype=np.int64)
    for c in range(P):
        groups = cores[c][0]
        for g, (base, end) in enumerate(groups):
            span = end - base
            lperm[c * NSH + base : c * NSH + end] = g * 128 + np.arange(span)
    assert (lperm >= 0).all()
    node_core = np.arange(N) // NSH
    perm = np.where(
        lperm < GPH,
        node_core * GPH + lperm,
        PHALF + node_core * GPH + (lperm - GPH),
    )

    gidx_all = np.zeros((P, G, 128, 2 * CAP // 16), dtype=np.int16)
    s_all = np.zeros((P, G, 128, NBLK * 128), dtype=np.float32)
    invd_all = np.zeros((P, G, 128, 1), dtype=np.float32)
    for c in range(P):
        groups, s_c, d_c = cores[c]
        ps_c = perm[s_c]
        isB = ps_c >= PHALF
        eA = np.nonzero(~isB)[0]
        eB = np.nonzero(isB)[0]
        dA = d_c[eA]
        dB = d_c[eB]
        for g in range(G):
            if g < len(groups):
                base, end = groups[g]
            else:
                base, end = 0, 0
            idxA = np.zeros(CAP, dtype=np.int16)
            idxB = np.zeros(CAP, dtype=np.int16)
            loA, hiA = np.searchsorted(dA, base), np.searchsorted(dA, end)
            loB, hiB = np.searchsorted(dB, base), np.searchsorted(dB, end)
            kA, kB = hiA - loA, hiB - loB
            assert kA <= CAP and kB <= CAP, (kA, kB)
            idxA[:kA] = ps_c[eA[loA:hiA]]
            idxB[:kB] = ps_c[eB[loB:hiB]] - PHALF
            gidx_all[c, g, :, : CAP // 16] = _pack_idx16(idxA)
            gidx_all[c, g, :, CAP // 16 :] = _pack_idx16(idxB)

            if g < len(groups):
                invd_all[c, g, : end - base, 0] = inv_deg[
                    c * NSH + base : c * NSH + end
                ]
            if kA:
                jj = np.arange(kA)
                dloc = d_c[eA[loA:hiA]] - base
                s_all[c, g, jj % 128, (jj // 128) * 128 + dloc] = 1.0
            if kB:
                jj = np.arange(kB)
                dloc = d_c[eB[loB:hiB]] - base
                s_all[c, g, jj % 128, (CAPB + jj // 128) * 128 + dloc] = 1.0

    return {
        "G": G,
        "perm": perm,
        "lperm": lperm,
        "cores": [g[0] for g in cores],
        "gidx": gidx_all,
        "stab": s_all.astype(mybir.dt.np(mybir.dt.float8e4)),
        "invd": invd_all,
    }


def _build_program(G):
    """Build + compile the single SPMD program (parametrized by group count)."""
    GP = G * 128
    GPH = GP // 2
    NP = P * GP
    PHALF = P * GPH
    nc = bacc.Bacc("TRN2", target_bir_lowering=False, debug=False, num_devices=P,
                   num_swdge_queues=4)
    f32, bf16, i16 = mybir.dt.float32, mybir.dt.bfloat16, mybir.dt.int16

    xh = nc.dram_tensor("xh", [NP, D], bf16, kind="ExternalInput")
    xsT = nc.dram_tensor("xsT", [128, 2, GP], bf16, kind="ExternalInput")
    wl = nc.dram_tensor("wl", [L, 2, 128, D], bf16, kind="ExternalInput")
    wr = nc.dram_tensor("wr", [L, 2, 128, D], bf16, kind="ExternalInput")
    bias = nc.dram_tensor("bias", [L, 2, 128, 1], f32, kind="ExternalInput")
    ident = nc.dram_tensor("ident", [128, 128], bf16, kind="ExternalInput")
    gidx = nc.dram_tensor("gidx", [G, 128, 2 * CAP // 16], i16, kind="ExternalInput")
    fp8 = mybir.dt.float8e4
    stab = nc.dram_tensor("stab", [G, 128, NBLK * 128], fp8, kind="ExternalInput")
    invd = nc.dram_tensor("invd", [G, 128, 1], f32, kind="ExternalInput")
    out = nc.dram_tensor("out", [GP, D], f32, kind="ExternalOutput")

    RELU = mybir.ActivationFunctionType.Relu
    IDENT = mybir.ActivationFunctionType.Identity

    with tile.TileContext(nc) as tc:
        with (
            tc.tile_pool(name="dram", bufs=1, space="DRAM") as dram,
            tc.tile_pool(name="const", bufs=1) as const,
            tc.tile_pool(name="xt", bufs=2) as xtp,
            tc.tile_pool(name="ga", bufs=5) as gap,
            tc.tile_pool(name="gb", bufs=5) as gbp,
            tc.tile_pool(name="sp", bufs=4) as sp,
            tc.tile_pool(name="gi", bufs=6) as gip,
            tc.tile_pool(name="stage", bufs=3) as stage,
            tc.tile_pool(name="pa", bufs=2, space="PSUM") as pap,
            tc.tile_pool(name="py", bufs=2, space="PSUM") as pyp,
            tc.tile_pool(name="pt", bufs=4, space="PSUM") as ptp,
        ):
            hshA_d = [
                dram.tile([GPH, D], bf16, tag=f"hsA{i}", name=f"hsA{i}")
                for i in range(2)
            ]
            hshB_d = [
                dram.tile([GPH, D], bf16, tag=f"hsB{i}", name=f"hsB{i}")
                for i in range(2)
            ]
            hfA_d = [
                dram.tile([PHALF, D], bf16, tag=f"hfa{i}", name=f"hfa{i}",
                          addr_space="Shared")
                for i in range(2)
            ]
            hfB_d = [
                dram.tile([NP - PHALF, D], bf16, tag=f"hfb{i}", name=f"hfb{i}",
                          addr_space="Shared")
                for i in range(2)
            ]

            # resident constants
            w_sb = {}
            for l in range(L):
                for k in range(2):
                    t = const.tile([128, D], bf16, tag=f"wl{l}{k}", name=f"wl{l}{k}")
                    nc.sync.dma_start(t[:], wl[l, k])
                    w_sb[("l", l, k)] = t
                    t = const.tile([128, D], bf16, tag=f"wr{l}{k}", name=f"wr{l}{k}")
                    nc.sync.dma_start(t[:], wr[l, k])
                    w_sb[("r", l, k)] = t
            b_sb = {}
            for l in range(L):
                for mh in range(2):
                    t = const.tile([128, 1], f32, tag=f"b{l}{mh}", name=f"b{l}{mh}")
                    nc.sync.dma_start(t[:], bias[l, mh])
                    b_sb[(l, mh)] = t
            id_sb = const.tile([128, 128], bf16, tag="ident", name="id_sb")
            nc.sync.dma_start(id_sb[:], ident[:])

            # persistent transposed-shard buffers (root path, feat-major)
            xt = [xtp.tile([128, 2, GP], bf16, tag="xt", name=f"xt{i}")
                  for i in range(2)]
            nc.sync.dma_start(xt[0][:], xsT[:])

            cur = 0
            for l in range(L):
                if l == 0:
                    srcA, srcB = xh[0:PHALF, :], xh[PHALF:NP, :]
                else:
                    srcA, srcB = hfA_d[l - 1][:], hfB_d[l - 1][:]
                for g in range(G):
                    gs = slice(g * 128, (g + 1) * 128)
                    gi = gip.tile([128, 2 * CAP // 16], i16, name="gi")
                    nc.sync.dma_start(gi[:], gidx[g])
                    st = sp.tile([128, NBLK * 128], fp8, name="st")
                    iv = gip.tile([128, 1], f32, name="iv", tag="iv")
                    nc.sync.dma_start(iv[:], invd[g])
                    nc.sync.dma_start(st[:], stab[g])
                    ga = gap.tile([128, CAPB, D], bf16, name="ga")
                    gb = gbp.tile([128, CAPB, D], bf16, name="gb")
                    nc.gpsimd.dma_gather(
                        ga[:], srcA, gi[:, 0 : CAP // 16],
                        CAP, CAP, D, queue_num=(2 * g) % 4,
                    )
                    nc.gpsimd.dma_gather(
                        gb[:], srcB, gi[:, CAP // 16 : 2 * CAP // 16],
                        CAP, CAP, D, queue_num=(2 * g + 1) % 4,
                    )

                    # segment-sum: agg[dst_slot, feat] in PSUM
                    pa = pap.tile([128, D], f32, name="pa")
                    for bb in range(NBLK):
                        gsrc = ga if bb < CAPB else gb
                        nc.tensor.matmul(
                            pa[:],
                            st[:, bb * 128 : (bb + 1) * 128],
                            gsrc[:, bb % CAPB, :],
                            start=(bb == 0),
                            stop=(bb == NBLK - 1),
                        )
                    ab = stage.tile([128, D], bf16, name="ab", tag="ab")
                    nc.scalar.activation(
                        ab[:], pa[:], mybir.ActivationFunctionType.Identity,
                        scale=iv[:],
                    )
                    # transpose agg to feat-major
                    aT = stage.tile([128, 2, 128], bf16, name="aT", tag="aT")
                    for k in range(2):
                        pt = ptp.tile([128, 128], bf16, name="pt")
                        nc.tensor.transpose(
                            pt[:], ab[:, k * 128 : (k + 1) * 128], id_sb[:]
                        )
                        nc.scalar.activation(
                            aT[:, k, :], pt[:],
                            mybir.ActivationFunctionType.Copy,
                        )

                    # dense: yT[mh] = sum_k Wl[k,mh]^T aggT[k] + Wr[k,mh]^T xT[k]
                    py = pyp.tile([128, 2, 128], f32, name="py")
                    for mh in range(2):
                        ms = slice(mh * 128, (mh + 1) * 128)
                        nc.tensor.matmul(py[:, mh, :], w_sb[("l", l, 0)][:, ms],
                                         aT[:, 0, :], start=True, stop=False)
                        nc.tensor.matmul(py[:, mh, :], w_sb[("l", l, 1)][:, ms],
                                         aT[:, 1, :], start=False, stop=False)
                        nc.tensor.matmul(py[:, mh, :], w_sb[("r", l, 0)][:, ms],
                                         xt[cur][:, 0, gs], start=False, stop=False)
                        nc.tensor.matmul(py[:, mh, :], w_sb[("r", l, 1)][:, ms],
                                         xt[cur][:, 1, gs], start=False, stop=True)
                    yT = stage.tile([128, 2, 128], bf16, name="yT", tag="yT")
                    for mh in range(2):
                        nc.scalar.activation(
                            yT[:, mh, :], py[:, mh, :],
                            RELU if l < L - 1 else IDENT,
                            bias=b_sb[(l, mh)][:],
                        )
                    if l < L - 1:
                        for mh in range(2):
                            nc.vector.tensor_copy(xt[1 - cur][:, mh, gs],
                                                  yT[:, mh, :])
                    # back to row-major for the halo replica / output
                    ydt = bf16 if l < L - 1 else f32
                    yr = stage.tile([128, D], ydt, name="yr",
                                    tag=f"yr{l == L - 1}")
                    for mh in range(2):
                        pt2 = ptp.tile([128, 128], bf16, name="pt2", tag="pt")
                        nc.tensor.transpose(pt2[:], yT[:, mh, :], id_sb[:])
                        nc.vector.tensor_copy(
                            yr[:, mh * 128 : (mh + 1) * 128], pt2[:]
                        )
                    if l < L - 1:
                        half = G // 2
                        tgt = hshA_d[l] if g < half else hshB_d[l]
                        go = (g if g < half else g - half) * 128
                        nc.sync.dma_start(tgt[go : go + 128, :], yr[:])
                    else:
                        nc.sync.dma_start(out[gs, :], yr[:])

                if l < L - 1:
                    nc.gpsimd.collective_compute(
                        "AllGather",
                        mybir.AluOpType.bypass,
                        replica_groups=[list(range(P))],
                        ins=[hshA_d[l][:]],
                        outs=[hfA_d[l][:]],
                    )
                    nc.gpsimd.collective_compute(
                        "AllGather",
                        mybir.AluOpType.bypass,
                        replica_groups=[list(range(P))],
                        ins=[hshB_d[l][:]],
                        outs=[hfB_d[l][:]],
                    )
                    cur = 1 - cur

    nc.compile()
    return nc


_CACHE = {}


def _get_program(G):
    if G not in _CACHE:
        _CACHE[G] = _build_program(G)
    return _CACHE[G]


LAST_EXEC_NS = None


def kernel(x, edge_index, Wl, Wr, b, _trace=False):
    global LAST_EXEC_NS
    x = np.asarray(x, dtype=np.float32)
    edge_index = np.asarray(edge_index)
    Wl = np.asarray(Wl, dtype=np.float32)
    Wr = np.asarray(Wr, dtype=np.float32)
    b = np.asarray(b, dtype=np.float32)

    pre = _preprocess(x, edge_index)
    G = pre["G"]
    GP = G * 128
    NP = P * GP
    nc = _get_program(G)

    # permuted (chunk-major) full replica and local-layout per-core rows
    xh32 = np.zeros((NP, D), dtype=np.float32)
    xh32[pre["perm"]] = x
    xh = xh32.astype(BF16)
    xloc = np.zeros((P, GP, D), dtype=np.float32)
    for c in range(P):
        xloc[c][pre["lperm"][c * NSH : (c + 1) * NSH]] = (
            x[c * NSH : (c + 1) * NSH]
        )

    wl_h = np.ascontiguousarray(Wl.reshape(L, 2, 128, D).astype(BF16))
    wr_h = np.ascontiguousarray(Wr.reshape(L, 2, 128, D).astype(BF16))
    b_h = np.ascontiguousarray(b.reshape(L, 2, 128, 1).astype(np.float32))
    id_h = np.eye(128, dtype=BF16)

    in_maps = []
    for c in range(P):
        xs = xloc[c]
        xsT = np.ascontiguousarray(
            xs.T.reshape(2, 128, GP).transpose(1, 0, 2).astype(BF16)
        )
        in_maps.append(
            {
                "xh": xh,
                "xsT": xsT,
                "wl": wl_h,
                "wr": wr_h,
                "bias": b_h,
                "ident": id_h,
                "gidx": pre["gidx"][c],
                "stab": pre["stab"][c],
                "invd": pre["invd"][c],
            }
        )

    res = run_bass_kernel_spmd(
        nc, in_maps, core_ids=list(range(P)), trace=bool(_trace)
    )
    LAST_EXEC_NS = res.exec_time_ns

    out_full = np.empty((N, D), dtype=np.float32)
    for c in range(P):
        out_full[c * NSH : (c + 1) * NSH] = res.results[c]["out"][
            pre["lperm"][c * NSH : (c + 1) * NSH]
        ]
    return out_full


# revision 15
# speedup vs baseline: 1.1268x; 1.1268x over previous
"""GraphSAGE (3-layer, mean-agg) on 8 Trainium2 NeuronCores.

Strategy (matches the sharding hint):
  - Nodes sharded by id range across 8 cores; 256x256 weights replicated;
    edges partitioned by destination-node owner.
  - Node space is PERMUTED into (core, group, slot) blocks: each core's
    6250 destinations are greedily packed into G groups of <=128 distinct
    dsts whose edges fit 1024 slots per src-half. Every group then owns a
    static, disjoint 128-row block of the layout — no scatter needed.
  - Per layer, per core, per group: dma_gather pulls h[src] rows (f32,
    1KB descriptors, 4 SWDGE queues) from a full replica of h in local
    HBM; host-built one-hot segment matrices S (inv_deg folded in) turn
    the segment-sum into TensorE matmuls accumulated in PSUM; the dense
    layer (y^T = Wl^T @ agg^T + Wr^T @ h^T + b, relu) is fused right
    after, with stationary weight tiles; y rows land at the group's
    static block in the shard buffer.
  - After layers 0 and 1 an AllGather rebuilds the full (permuted)
    replica. Output is un-permuted on the host.
"""

import sys

sys.path.insert(0, "/opt/trn_rl_repo")

import numpy as np
import ml_dtypes

import concourse.bass as bass
import concourse.bacc as bacc
import concourse.tile as tile
import concourse.mybir as mybir
from concourse.bass_utils import run_bass_kernel_spmd

BF16 = ml_dtypes.bfloat16

N = 50000
E = 800000
D = 256
L = 3
P = 8
NSH = N // P            # 6250 nodes per core
CAPB = 8                # gather blocks (of 128 edges) per src-half per group
CAP = CAPB * 128        # 1024 edge slots per src-half per group
NBLK = 2 * CAPB         # 16 segment matmul blocks per group


def _pack_idx16(idx):
    """Pack idx list (len multiple of 16) into [128, len/16] int16 layout:
    slot j -> [j % 16, j // 16], replicated to all 8 Q7-core stripes."""
    n = idx.shape[0]
    return np.tile(idx.reshape(n // 16, 16).T, (8, 1)).astype(np.int16)


def _preprocess(x, edge_index):
    """Group edges by dst windows; build permuted layout + gather/segment
    tables. Returns dict of host arrays + layout info."""
    src = edge_index[0].astype(np.int64)
    dst = edge_index[1].astype(np.int64)
    deg = np.bincount(dst, minlength=N).astype(np.float64)
    inv_deg = (1.0 / np.maximum(deg, 1.0)).astype(np.float32)

    cores = []
    for c in range(P):
        lo, hi = c * NSH, (c + 1) * NSH
        m = (dst >= lo) & (dst < hi)
        s_c = src[m]
        d_c = dst[m] - lo
        order = np.argsort(d_c, kind="stable")
        s_c, d_c = s_c[order], d_c[order]
        isB = s_c >= N // 2
        degA = np.bincount(d_c[~isB], minlength=NSH)
        degB = np.bincount(d_c[isB], minlength=NSH)
        assert degA.max() <= CAP and degB.max() <= CAP

        groups = []  # (base, end)
        base, ca, cb = 0, 0, 0
        for dd in range(NSH):
            da, db = degA[dd], degB[dd]
            if (ca + da > CAP) or (cb + db > CAP) or (dd - base >= 128):
                groups.append((base, dd))
                base, ca, cb = dd, 0, 0
            ca += da
            cb += db
        groups.append((base, NSH))
        cores.append((groups, s_c, d_c, isB))

    G = max(len(g[0]) for g in cores)
    GP = G * 128            # permuted rows per core
    NP = P * GP             # total permuted rows
    PHALF = NP // 2         # src-half split (core-major, so = id split)
    assert PHALF < 32768

    # node id -> permuted row
    perm = np.full(N, -1, dtype=np.int64)
    for c in range(P):
        groups = cores[c][0]
        for g, (base, end) in enumerate(groups):
            span = end - base
            perm[c * NSH + base : c * NSH + end] = (
                c * GP + g * 128 + np.arange(span)
            )
    assert (perm >= 0).all()

    gidx_all = np.zeros((P, G, 128, 2 * CAP // 16), dtype=np.int16)
    s_all = np.zeros((P, G, 128, NBLK * 128), dtype=np.float32)
    invd_all = np.zeros((P, G, 128, 1), dtype=np.float32)
    p_src = perm[src]
    for c in range(P):
        groups, s_c, d_c, isB = cores[c]
        ps_c = perm[s_c]
        eA = np.nonzero(~isB)[0]
        eB = np.nonzero(isB)[0]
        dA = d_c[eA]
        dB = d_c[eB]
        for g in range(G):
            if g < len(groups):
                base, end = groups[g]
            else:
                base, end = 0, 0
            idxA = np.zeros(CAP, dtype=np.int16)
            idxB = np.zeros(CAP, dtype=np.int16)
            loA, hiA = np.searchsorted(dA, base), np.searchsorted(dA, end)
            loB, hiB = np.searchsorted(dB, base), np.searchsorted(dB, end)
            kA, kB = hiA - loA, hiB - loB
            assert kA <= CAP and kB <= CAP
            idxA[:kA] = ps_c[eA[loA:hiA]]
            idxB[:kB] = ps_c[eB[loB:hiB]] - PHALF
            gidx_all[c, g, :, : CAP // 16] = _pack_idx16(idxA)
            gidx_all[c, g, :, CAP // 16 :] = _pack_idx16(idxB)

            if g < len(groups):
                invd_all[c, g, : end - base, 0] = inv_deg[
                    c * NSH + base : c * NSH + end
                ]
            if kA:
                jj = np.arange(kA)
                dloc = d_c[eA[loA:hiA]] - base
                s_all[c, g, jj % 128, (jj // 128) * 128 + dloc] = 1.0
            if kB:
                jj = np.arange(kB)
                dloc = d_c[eB[loB:hiB]] - base
                s_all[c, g, jj % 128, (CAPB + jj // 128) * 128 + dloc] = 1.0

    return {
        "G": G,
        "perm": perm,
        "cores": [g[0] for g in cores],
        "gidx": gidx_all,
        "stab": s_all.astype(mybir.dt.np(mybir.dt.float8e4)),
        "invd": invd_all,
    }


def _build_program(G):
    """Build + compile the single SPMD program (parametrized by group count)."""
    GP = G * 128
    NP = P * GP
    PHALF = NP // 2
    nc = bacc.Bacc("TRN2", target_bir_lowering=False, debug=False, num_devices=P,
                   num_swdge_queues=4)
    f32, bf16, i16 = mybir.dt.float32, mybir.dt.bfloat16, mybir.dt.int16

    xh = nc.dram_tensor("xh", [NP, D], bf16, kind="ExternalInput")
    xsT = nc.dram_tensor("xsT", [128, 2, GP], bf16, kind="ExternalInput")
    wl = nc.dram_tensor("wl", [L, 2, 128, D], bf16, kind="ExternalInput")
    wr = nc.dram_tensor("wr", [L, 2, 128, D], bf16, kind="ExternalInput")
    bias = nc.dram_tensor("bias", [L, 2, 128, 1], f32, kind="ExternalInput")
    ident = nc.dram_tensor("ident", [128, 128], bf16, kind="ExternalInput")
    gidx = nc.dram_tensor("gidx", [G, 128, 2 * CAP // 16], i16, kind="ExternalInput")
    fp8 = mybir.dt.float8e4
    stab = nc.dram_tensor("stab", [G, 128, NBLK * 128], fp8, kind="ExternalInput")
    invd = nc.dram_tensor("invd", [G, 128, 1], f32, kind="ExternalInput")
    out = nc.dram_tensor("out", [GP, D], f32, kind="ExternalOutput")

    RELU = mybir.ActivationFunctionType.Relu
    IDENT = mybir.ActivationFunctionType.Identity

    with tile.TileContext(nc) as tc:
        with (
            tc.tile_pool(name="dram", bufs=1, space="DRAM") as dram,
            tc.tile_pool(name="const", bufs=1) as const,
            tc.tile_pool(name="xt", bufs=2) as xtp,
            tc.tile_pool(name="ga", bufs=5) as gap,
            tc.tile_pool(name="gb", bufs=5) as gbp,
            tc.tile_pool(name="sp", bufs=4) as sp,
            tc.tile_pool(name="gi", bufs=6) as gip,
            tc.tile_pool(name="stage", bufs=3) as stage,
            tc.tile_pool(name="pa", bufs=2, space="PSUM") as pap,
            tc.tile_pool(name="py", bufs=2, space="PSUM") as pyp,
            tc.tile_pool(name="pt", bufs=4, space="PSUM") as ptp,
        ):
            hshard_d = [
                dram.tile([GP, D], bf16, tag=f"hsh{i}", name=f"hsh{i}")
                for i in range(2)
            ]
            hfull_d = [
                dram.tile([NP, D], bf16, tag=f"hfl{i}", name=f"hfl{i}",
                          addr_space="Shared")
                for i in range(2)
            ]

            # resident constants
            w_sb = {}
            for l in range(L):
                for k in range(2):
                    t = const.tile([128, D], bf16, tag=f"wl{l}{k}", name=f"wl{l}{k}")
                    nc.sync.dma_start(t[:], wl[l, k])
                    w_sb[("l", l, k)] = t
                    t = const.tile([128, D], bf16, tag=f"wr{l}{k}", name=f"wr{l}{k}")
                    nc.sync.dma_start(t[:], wr[l, k])
                    w_sb[("r", l, k)] = t
            b_sb = {}
            for l in range(L):
                for mh in range(2):
                    t = const.tile([128, 1], f32, tag=f"b{l}{mh}", name=f"b{l}{mh}")
                    nc.sync.dma_start(t[:], bias[l, mh])
                    b_sb[(l, mh)] = t
            id_sb = const.tile([128, 128], bf16, tag="ident", name="id_sb")
            nc.sync.dma_start(id_sb[:], ident[:])

            # persistent transposed-shard buffers (root path, feat-major)
            xt = [xtp.tile([128, 2, GP], bf16, tag="xt", name=f"xt{i}")
                  for i in range(2)]
            nc.sync.dma_start(xt[0][:], xsT[:])

            cur = 0
            for l in range(L):
                src_t = xh if l == 0 else hfull_d[l - 1]
                for g in range(G):
                    gs = slice(g * 128, (g + 1) * 128)
                    gi = gip.tile([128, 2 * CAP // 16], i16, name="gi")
                    nc.sync.dma_start(gi[:], gidx[g])
                    st = sp.tile([128, NBLK * 128], fp8, name="st")
                    iv = gip.tile([128, 1], f32, name="iv", tag="iv")
                    nc.sync.dma_start(iv[:], invd[g])
                    nc.sync.dma_start(st[:], stab[g])
                    ga = gap.tile([128, CAPB, D], bf16, name="ga")
                    gb = gbp.tile([128, CAPB, D], bf16, name="gb")
                    nc.gpsimd.dma_gather(
                        ga[:], src_t[0:PHALF, :], gi[:, 0 : CAP // 16],
                        CAP, CAP, D, queue_num=(2 * g) % 4,
                    )
                    nc.gpsimd.dma_gather(
                        gb[:], src_t[PHALF:NP, :], gi[:, CAP // 16 : 2 * CAP // 16],
                        CAP, CAP, D, queue_num=(2 * g + 1) % 4,
                    )

                    # segment-sum: agg[dst_slot, feat] in PSUM
                    pa = pap.tile([128, D], f32, name="pa")
                    for bb in range(NBLK):
                        gsrc = ga if bb < CAPB else gb
                        nc.tensor.matmul(
                            pa[:],
                            st[:, bb * 128 : (bb + 1) * 128],
                            gsrc[:, bb % CAPB, :],
                            start=(bb == 0),
                            stop=(bb == NBLK - 1),
                        )
                    ab = stage.tile([128, D], bf16, name="ab", tag="ab")
                    nc.scalar.activation(
                        ab[:], pa[:], mybir.ActivationFunctionType.Identity,
                        scale=iv[:],
                    )
                    # transpose agg to feat-major
                    aT = stage.tile([128, 2, 128], bf16, name="aT", tag="aT")
                    for k in range(2):
                        pt = ptp.tile([128, 128], bf16, name="pt")
                        nc.tensor.transpose(
                            pt[:], ab[:, k * 128 : (k + 1) * 128], id_sb[:]
                        )
                        nc.scalar.activation(
                            aT[:, k, :], pt[:],
                            mybir.ActivationFunctionType.Copy,
                        )

                    # dense: yT[mh] = sum_k Wl[k,mh]^T aggT[k] + Wr[k,mh]^T xT[k]
                    py = pyp.tile([128, 2, 128], f32, name="py")
                    for mh in range(2):
                        ms = slice(mh * 128, (mh + 1) * 128)
                        nc.tensor.matmul(py[:, mh, :], w_sb[("l", l, 0)][:, ms],
                                         aT[:, 0, :], start=True, stop=False)
                        nc.tensor.matmul(py[:, mh, :], w_sb[("l", l, 1)][:, ms],
                                         aT[:, 1, :], start=False, stop=False)
                        nc.tensor.matmul(py[:, mh, :], w_sb[("r", l, 0)][:, ms],
                                         xt[cur][:, 0, gs], start=False, stop=False)
                        nc.tensor.matmul(py[:, mh, :], w_sb[("r", l, 1)][:, ms],
                                         xt[cur][:, 1, gs], start=False, stop=True)
                    yT = stage.tile([128, 2, 128], bf16, name="yT", tag="yT")
                    for mh in range(2):
                        nc.scalar.activation(
                            yT[:, mh, :], py[:, mh, :],
                            RELU if l < L - 1 else IDENT,
                            bias=b_sb[(l, mh)][:],
                        )
                    if l < L - 1:
                        for mh in range(2):
                            nc.vector.tensor_copy(xt[1 - cur][:, mh, gs],
                                                  yT[:, mh, :])
                    # back to row-major for the halo replica / output
                    ydt = bf16 if l < L - 1 else f32
                    yr = stage.tile([128, D], ydt, name="yr",
                                    tag=f"yr{l == L - 1}")
                    for mh in range(2):
                        pt2 = ptp.tile([128, 128], bf16, name="pt2", tag="pt")
                        nc.tensor.transpose(pt2[:], yT[:, mh, :], id_sb[:])
                        nc.vector.tensor_copy(
                            yr[:, mh * 128 : (mh + 1) * 128], pt2[:]
                        )
                    if l < L - 1:
                        nc.sync.dma_start(hshard_d[l][gs, :], yr[:])
                    else:
                        nc.sync.dma_start(out[gs, :], yr[:])

                if l < L - 1:
                    nc.gpsimd.collective_compute(
                        "AllGather",
                        mybir.AluOpType.bypass,
                        replica_groups=[list(range(P))],
                        ins=[hshard_d[l][:]],
                        outs=[hfull_d[l][:]],
                    )
                    cur = 1 - cur

    nc.compile()
    return nc


_CACHE = {}


def _get_program(G):
    if G not in _CACHE:
        _CACHE[G] = _build_program(G)
    return _CACHE[G]


LAST_EXEC_NS = None


def kernel(x, edge_index, Wl, Wr, b, _trace=False):
    global LAST_EXEC_NS
    x = np.asarray(x, dtype=np.float32)
    edge_index = np.asarray(edge_index)
    Wl = np.asarray(Wl, dtype=np.float32)
    Wr = np.asarray(Wr, dtype=np.float32)
    b = np.asarray(b, dtype=np.float32)

    pre = _preprocess(x, edge_index)
    G = pre["G"]
    GP = G * 128
    NP = P * GP
    nc = _get_program(G)

    # permuted full replica
    xh32 = np.zeros((NP, D), dtype=np.float32)
    xh32[pre["perm"]] = x
    xh = xh32.astype(BF16)

    wl_h = np.ascontiguousarray(Wl.reshape(L, 2, 128, D).astype(BF16))
    wr_h = np.ascontiguousarray(Wr.reshape(L, 2, 128, D).astype(BF16))
    b_h = np.ascontiguousarray(b.reshape(L, 2, 128, 1).astype(np.float32))
    id_h = np.eye(128, dtype=BF16)

    in_maps = []
    for c in range(P):
        xs = xh32[c * GP : (c + 1) * GP]
        xsT = np.ascontiguousarray(
            xs.T.reshape(2, 128, GP).transpose(1, 0, 2).astype(BF16)
        )
        in_maps.append(
            {
                "xh": xh,
                "xsT": xsT,
                "wl": wl_h,
                "wr": wr_h,
                "bias": b_h,
                "ident": id_h,
                "gidx": pre["gidx"][c],
                "stab": pre["stab"][c],
                "invd": pre["invd"][c],
            }
        )

    res = run_bass_kernel_spmd(
        nc, in_maps, core_ids=list(range(P)), trace=bool(_trace)
    )
    LAST_EXEC_NS = res.exec_time_ns

    out_full = np.empty((N, D), dtype=np.float32)
    outs = np.concatenate([res.results[c]["out"] for c in range(P)], axis=0)
    out_full[:] = outs[pre["perm"]]
    return out_full


# revision 16
# speedup vs baseline: 1.3057x; 1.1587x over previous
"""GraphSAGE (3-layer, mean-agg) on 8 Trainium2 NeuronCores.

Strategy (matches the sharding hint):
  - Nodes sharded by id range across 8 cores; 256x256 weights replicated;
    edges partitioned by destination-node owner.
  - Node space is PERMUTED into (core, group, slot) blocks: each core's
    6250 destinations are greedily packed into G groups of <=128 distinct
    dsts whose edges fit 1024 slots per src-half. Every group then owns a
    static, disjoint 128-row block of the layout — no scatter needed.
  - Per layer, per core, per group: dma_gather pulls h[src] rows (f32,
    1KB descriptors, 4 SWDGE queues) from a full replica of h in local
    HBM; host-built one-hot segment matrices S (inv_deg folded in) turn
    the segment-sum into TensorE matmuls accumulated in PSUM; the dense
    layer (y^T = Wl^T @ agg^T + Wr^T @ h^T + b, relu) is fused right
    after, with stationary weight tiles; y rows land at the group's
    static block in the shard buffer.
  - After layers 0 and 1 an AllGather rebuilds the full (permuted)
    replica. Output is un-permuted on the host.
"""

import sys

sys.path.insert(0, "/opt/trn_rl_repo")

import numpy as np
import ml_dtypes

import concourse.bass as bass
import concourse.bacc as bacc
import concourse.tile as tile
import concourse.mybir as mybir
from concourse.bass_utils import run_bass_kernel_spmd

BF16 = ml_dtypes.bfloat16

N = 50000
E = 800000
D = 256
L = 3
P = 8
NSH = N // P            # 6250 nodes per core
CAPB = 8                # gather blocks (of 128 edges) per src-half per group
CAP = CAPB * 128        # 1024 edge slots per src-half per group
NBLK = 2 * CAPB         # 16 segment matmul blocks per group


def _pack_idx16(idx):
    """Pack idx list (len multiple of 16) into [128, len/16] int16 layout:
    slot j -> [j % 16, j // 16], replicated to all 8 Q7-core stripes."""
    n = idx.shape[0]
    return np.tile(idx.reshape(n // 16, 16).T, (8, 1)).astype(np.int16)


def _preprocess(x, edge_index):
    """Group edges by dst windows; build permuted layout + gather/segment
    tables. Returns dict of host arrays + layout info."""
    src = edge_index[0].astype(np.int64)
    dst = edge_index[1].astype(np.int64)
    deg = np.bincount(dst, minlength=N).astype(np.float64)
    inv_deg = (1.0 / np.maximum(deg, 1.0)).astype(np.float32)

    cores = []
    for c in range(P):
        lo, hi = c * NSH, (c + 1) * NSH
        m = (dst >= lo) & (dst < hi)
        s_c = src[m]
        d_c = dst[m] - lo
        order = np.argsort(d_c, kind="stable")
        s_c, d_c = s_c[order], d_c[order]
        isB = s_c >= N // 2
        degA = np.bincount(d_c[~isB], minlength=NSH)
        degB = np.bincount(d_c[isB], minlength=NSH)
        assert degA.max() <= CAP and degB.max() <= CAP

        groups = []  # (base, end)
        base, ca, cb = 0, 0, 0
        for dd in range(NSH):
            da, db = degA[dd], degB[dd]
            if (ca + da > CAP) or (cb + db > CAP) or (dd - base >= 128):
                groups.append((base, dd))
                base, ca, cb = dd, 0, 0
            ca += da
            cb += db
        groups.append((base, NSH))
        cores.append((groups, s_c, d_c, isB))

    G = max(len(g[0]) for g in cores)
    GP = G * 128            # permuted rows per core
    NP = P * GP             # total permuted rows
    PHALF = NP // 2         # src-half split (core-major, so = id split)
    assert PHALF < 32768

    # node id -> permuted row
    perm = np.full(N, -1, dtype=np.int64)
    for c in range(P):
        groups = cores[c][0]
        for g, (base, end) in enumerate(groups):
            span = end - base
            perm[c * NSH + base : c * NSH + end] = (
                c * GP + g * 128 + np.arange(span)
            )
    assert (perm >= 0).all()

    gidx_all = np.zeros((P, G, 128, 2 * CAP // 16), dtype=np.int16)
    s_all = np.zeros((P, G, 128, NBLK * 128), dtype=np.float32)
    invd_all = np.zeros((P, G, 128, 1), dtype=np.float32)
    p_src = perm[src]
    for c in range(P):
        groups, s_c, d_c, isB = cores[c]
        ps_c = perm[s_c]
        eA = np.nonzero(~isB)[0]
        eB = np.nonzero(isB)[0]
        dA = d_c[eA]
        dB = d_c[eB]
        for g in range(G):
            if g < len(groups):
                base, end = groups[g]
            else:
                base, end = 0, 0
            idxA = np.zeros(CAP, dtype=np.int16)
            idxB = np.zeros(CAP, dtype=np.int16)
            loA, hiA = np.searchsorted(dA, base), np.searchsorted(dA, end)
            loB, hiB = np.searchsorted(dB, base), np.searchsorted(dB, end)
            kA, kB = hiA - loA, hiB - loB
            assert kA <= CAP and kB <= CAP
            idxA[:kA] = ps_c[eA[loA:hiA]]
            idxB[:kB] = ps_c[eB[loB:hiB]] - PHALF
            gidx_all[c, g, :, : CAP // 16] = _pack_idx16(idxA)
            gidx_all[c, g, :, CAP // 16 :] = _pack_idx16(idxB)

            if g < len(groups):
                invd_all[c, g, : end - base, 0] = inv_deg[
                    c * NSH + base : c * NSH + end
                ]
            if kA:
                jj = np.arange(kA)
                dloc = d_c[eA[loA:hiA]] - base
                s_all[c, g, jj % 128, (jj // 128) * 128 + dloc] = 1.0
            if kB:
                jj = np.arange(kB)
                dloc = d_c[eB[loB:hiB]] - base
                s_all[c, g, jj % 128, (CAPB + jj // 128) * 128 + dloc] = 1.0

    return {
        "G": G,
        "perm": perm,
        "cores": [g[0] for g in cores],
        "gidx": gidx_all,
        "stab": s_all.astype(mybir.dt.np(mybir.dt.float8e4)),
        "invd": invd_all,
    }


def _build_program(G):
    """Build + compile the single SPMD program (parametrized by group count)."""
    GP = G * 128
    NP = P * GP
    PHALF = NP // 2
    nc = bacc.Bacc("TRN2", target_bir_lowering=False, debug=False, num_devices=P,
                   num_swdge_queues=4)
    f32, bf16, i16 = mybir.dt.float32, mybir.dt.bfloat16, mybir.dt.int16

    xh = nc.dram_tensor("xh", [NP, D], bf16, kind="ExternalInput")
    xsT = nc.dram_tensor("xsT", [128, 2, GP], bf16, kind="ExternalInput")
    wl = nc.dram_tensor("wl", [L, 2, 128, D], bf16, kind="ExternalInput")
    wr = nc.dram_tensor("wr", [L, 2, 128, D], bf16, kind="ExternalInput")
    bias = nc.dram_tensor("bias", [L, 2, 128, 1], f32, kind="ExternalInput")
    ident = nc.dram_tensor("ident", [128, 128], bf16, kind="ExternalInput")
    gidx = nc.dram_tensor("gidx", [G, 128, 2 * CAP // 16], i16, kind="ExternalInput")
    fp8 = mybir.dt.float8e4
    stab = nc.dram_tensor("stab", [G, 128, NBLK * 128], fp8, kind="ExternalInput")
    invd = nc.dram_tensor("invd", [G, 128, 1], f32, kind="ExternalInput")
    out = nc.dram_tensor("out", [GP, D], f32, kind="ExternalOutput")

    RELU = mybir.ActivationFunctionType.Relu
    IDENT = mybir.ActivationFunctionType.Identity

    with tile.TileContext(nc) as tc:
        with (
            tc.tile_pool(name="dram", bufs=1, space="DRAM") as dram,
            tc.tile_pool(name="const", bufs=1) as const,
            tc.tile_pool(name="xt", bufs=2) as xtp,
            tc.tile_pool(name="ga", bufs=7) as gap,
            tc.tile_pool(name="gb", bufs=7) as gbp,
            tc.tile_pool(name="sp", bufs=4) as sp,
            tc.tile_pool(name="gi", bufs=6) as gip,
            tc.tile_pool(name="stage", bufs=4) as stage,
            tc.tile_pool(name="pa", bufs=2, space="PSUM") as pap,
            tc.tile_pool(name="py", bufs=2, space="PSUM") as pyp,
            tc.tile_pool(name="pt", bufs=4, space="PSUM") as ptp,
        ):
            hshard_d = [
                dram.tile([GP, D], bf16, tag=f"hsh{i}", name=f"hsh{i}")
                for i in range(2)
            ]
            hfull_d = [
                dram.tile([NP, D], bf16, tag=f"hfl{i}", name=f"hfl{i}",
                          addr_space="Shared")
                for i in range(2)
            ]

            # resident constants
            w_sb = {}
            for l in range(L):
                for k in range(2):
                    t = const.tile([128, D], bf16, tag=f"wl{l}{k}", name=f"wl{l}{k}")
                    nc.sync.dma_start(t[:], wl[l, k])
                    w_sb[("l", l, k)] = t
                    t = const.tile([128, D], bf16, tag=f"wr{l}{k}", name=f"wr{l}{k}")
                    nc.sync.dma_start(t[:], wr[l, k])
                    w_sb[("r", l, k)] = t
            b_sb = {}
            for l in range(L):
                for mh in range(2):
                    t = const.tile([128, 1], f32, tag=f"b{l}{mh}", name=f"b{l}{mh}")
                    nc.sync.dma_start(t[:], bias[l, mh])
                    b_sb[(l, mh)] = t
            id_sb = const.tile([128, 128], bf16, tag="ident", name="id_sb")
            nc.sync.dma_start(id_sb[:], ident[:])

            # persistent transposed-shard buffers (root path, feat-major)
            xt = [xtp.tile([128, 2, GP], bf16, tag="xt", name=f"xt{i}")
                  for i in range(2)]
            nc.sync.dma_start(xt[0][:], xsT[:])

            cur = 0
            for l in range(L):
                src_t = xh if l == 0 else hfull_d[l - 1]
                for g in range(G):
                    gs = slice(g * 128, (g + 1) * 128)
                    gi = gip.tile([128, 2 * CAP // 16], i16, name="gi")
                    nc.sync.dma_start(gi[:], gidx[g])
                    st = sp.tile([128, NBLK * 128], fp8, name="st")
                    iv = gip.tile([128, 1], f32, name="iv", tag="iv")
                    nc.sync.dma_start(iv[:], invd[g])
                    nc.sync.dma_start(st[:], stab[g])
                    ga = gap.tile([128, CAPB, D], bf16, name="ga")
                    gb = gbp.tile([128, CAPB, D], bf16, name="gb")
                    nc.gpsimd.dma_gather(
                        ga[:], src_t[0:PHALF, :], gi[:, 0 : CAP // 16],
                        CAP, CAP, D, queue_num=(2 * g) % 4,
                    )
                    nc.gpsimd.dma_gather(
                        gb[:], src_t[PHALF:NP, :], gi[:, CAP // 16 : 2 * CAP // 16],
                        CAP, CAP, D, queue_num=(2 * g + 1) % 4,
                    )

                    # segment-sum: agg[dst_slot, feat] in PSUM
                    pa = pap.tile([128, D], f32, name="pa")
                    for bb in range(NBLK):
                        gsrc = ga if bb < CAPB else gb
                        nc.tensor.matmul(
                            pa[:],
                            st[:, bb * 128 : (bb + 1) * 128],
                            gsrc[:, bb % CAPB, :],
                            start=(bb == 0),
                            stop=(bb == NBLK - 1),
                        )
                    ab = stage.tile([128, D], bf16, name="ab", tag="ab")
                    nc.scalar.activation(
                        ab[:], pa[:], mybir.ActivationFunctionType.Identity,
                        scale=iv[:],
                    )
                    # transpose agg to feat-major
                    aT = stage.tile([128, 2, 128], bf16, name="aT", tag="aT")
                    for k in range(2):
                        pt = ptp.tile([128, 128], bf16, name="pt")
                        nc.tensor.transpose(
                            pt[:], ab[:, k * 128 : (k + 1) * 128], id_sb[:]
                        )
                        nc.scalar.activation(
                            aT[:, k, :], pt[:],
                            mybir.ActivationFunctionType.Copy,
                        )

                    # dense: yT[mh] = sum_k Wl[k,mh]^T aggT[k] + Wr[k,mh]^T xT[k]
                    py = pyp.tile([128, 2, 128], f32, name="py")
                    for mh in range(2):
                        ms = slice(mh * 128, (mh + 1) * 128)
                        nc.tensor.matmul(py[:, mh, :], w_sb[("l", l, 0)][:, ms],
                                         aT[:, 0, :], start=True, stop=False)
                        nc.tensor.matmul(py[:, mh, :], w_sb[("l", l, 1)][:, ms],
                                         aT[:, 1, :], start=False, stop=False)
                        nc.tensor.matmul(py[:, mh, :], w_sb[("r", l, 0)][:, ms],
                                         xt[cur][:, 0, gs], start=False, stop=False)
                        nc.tensor.matmul(py[:, mh, :], w_sb[("r", l, 1)][:, ms],
                                         xt[cur][:, 1, gs], start=False, stop=True)
                    yT = stage.tile([128, 2, 128], bf16, name="yT", tag="yT")
                    for mh in range(2):
                        nc.scalar.activation(
                            yT[:, mh, :], py[:, mh, :],
                            RELU if l < L - 1 else IDENT,
                            bias=b_sb[(l, mh)][:],
                        )
                    if l < L - 1:
                        for mh in range(2):
                            nc.vector.tensor_copy(xt[1 - cur][:, mh, gs],
                                                  yT[:, mh, :])
                    # back to row-major for the halo replica / output
                    ydt = bf16 if l < L - 1 else f32
                    yr = stage.tile([128, D], ydt, name="yr",
                                    tag=f"yr{l == L - 1}")
                    for mh in range(2):
                        pt2 = ptp.tile([128, 128], bf16, name="pt2", tag="pt")
                        nc.tensor.transpose(pt2[:], yT[:, mh, :], id_sb[:])
                        nc.vector.tensor_copy(
                            yr[:, mh * 128 : (mh + 1) * 128], pt2[:]
                        )
                    if l < L - 1:
                        nc.sync.dma_start(hshard_d[l][gs, :], yr[:])
                    else:
                        nc.sync.dma_start(out[gs, :], yr[:])

                if l < L - 1:
                    nc.gpsimd.collective_compute(
                        "AllGather",
                        mybir.AluOpType.bypass,
                        replica_groups=[list(range(P))],
                        ins=[hshard_d[l][:]],
                        outs=[hfull_d[l][:]],
                    )
                    cur = 1 - cur

    nc.compile()
    return nc


_CACHE = {}


def _get_program(G):
    if G not in _CACHE:
        _CACHE[G] = _build_program(G)
    return _CACHE[G]


LAST_EXEC_NS = None


def kernel(x, edge_index, Wl, Wr, b, _trace=False):
    global LAST_EXEC_NS
    x = np.asarray(x, dtype=np.float32)
    edge_index = np.asarray(edge_index)
    Wl = np.asarray(Wl, dtype=np.float32)
    Wr = np.asarray(Wr, dtype=np.float32)
    b = np.asarray(b, dtype=np.float32)

    pre = _preprocess(x, edge_index)
    G = pre["G"]
    GP = G * 128
    NP = P * GP
    nc = _get_program(G)

    # permuted full replica
    xh32 = np.zeros((NP, D), dtype=np.float32)
    xh32[pre["perm"]] = x
    xh = xh32.astype(BF16)

    wl_h = np.ascontiguousarray(Wl.reshape(L, 2, 128, D).astype(BF16))
    wr_h = np.ascontiguousarray(Wr.reshape(L, 2, 128, D).astype(BF16))
    b_h = np.ascontiguousarray(b.reshape(L, 2, 128, 1).astype(np.float32))
    id_h = np.eye(128, dtype=BF16)

    in_maps = []
    for c in range(P):
        xs = xh32[c * GP : (c + 1) * GP]
        xsT = np.ascontiguousarray(
            xs.T.reshape(2, 128, GP).transpose(1, 0, 2).astype(BF16)
        )
        in_maps.append(
            {
                "xh": xh,
                "xsT": xsT,
                "wl": wl_h,
                "wr": wr_h,
                "bias": b_h,
                "ident": id_h,
                "gidx": pre["gidx"][c],
                "stab": pre["stab"][c],
                "invd": pre["invd"][c],
            }
        )

    res = run_bass_kernel_spmd(
        nc, in_maps, core_ids=list(range(P)), trace=bool(_trace)
    )
    LAST_EXEC_NS = res.exec_time_ns

    out_full = np.empty((N, D), dtype=np.float32)
    outs = np.concatenate([res.results[c]["out"] for c in range(P)], axis=0)
    out_full[:] = outs[pre["perm"]]
    return out_full
